# revision 77
# baseline (speedup 1.0000x reference)
"""Multi-head causal attention (B=4, S=2048, D=1024, H=16) on 8 trn2 NeuronCores.

Sharding: core = (batch b, head-group g), g in 0..1; each core computes heads
g*8..g*8+7 of batch b end-to-end plus its partial output projection; host sums
the two partials per batch and adds (bo + bv @ Wo.T) (the v-bias folds into
the host-side bias since softmax weights sum to 1).

Per-core dataflow, shaped for a PE whose matmul time is
(output free size) x (cycles/row of the moving dtype):
  QKV proj   compensated fp8 DoubleRow: x ~ (x8+dx8)/16, w ~ (w8+dw8)/256
             (host-split pairs), 3 product terms at a uniform 4096x PSUM
             scale -> 0.75x the bf16 row count at ~bf16 accuracy; the
             psum->sbuf copy rescales by 1/4096 and adds the q/k biases.
  scores     q/k stored fp8e4 in a DoubleRow layout (partition =
             (head%4)*32 + dk%32, k-tile dim = dk//32): one 0.5-cyc/row
             matmul per (head, k-tile), causal-tight sq windows.
  exp        Scalar-engine Exp into bf16 eg tiles; a measured-safe subset
             of late-block tiles computes exp on DVE instead via an int16
             Schraudolph (bf16-bits magic constant), balancing engines.
  attn@v     FLIPPED: stationary eg [sk, sq-128], moving vhx [sk, 65]
             (bf16) -> 65 rows/matmul; softmax denominator rides in column
             64 (ones column of vhx); causal masks only touch the 16
             true-diagonal 128x128 triangles.
  normalize  per-sq-partition reciprocal [128,4] + one broadcast multiply.
  transpose  PE transpose attn_out [sq,c]->[c,sq] against a bf16 identity.
  out proj   stationary wo bf16, moving outT bf16; last block's psum
             copies run on the (then idle) Scalar engine.

Measured on the harness input: ~200us/core (TimelineSim), rel err 7.8e-3
vs the fp32 reference (gate 2e-2).
"""

import sys

sys.path.insert(0, "/opt/trn_rl_repo")

import numpy as np

B, S, D, H, DK = 4, 2048, 1024, 16, 64
NCORES = 8
CPG = 512          # channels per core (8 heads)
HPC = 8            # heads per core
NB = 4             # sq blocks of 512
SQB = 512
NDT = D // 128     # 8 d-tiles
NCT = CPG // 128   # 4 c-tiles per core (= head pairs)
NST = S // 128     # 16 s-tiles

_PROGRAM = None

# exp tiles offloaded from Scalar via bf16 Schraudolph; returns None (keep
# Scalar Exp), 'v' (DVE) or 'p' (gpsimd). Subset must have measured-good
# end-to-end error.
def SCHR_SEL(blk, j):
    if blk == 3 and j % 2 == 1:
        return 'v'
    if blk == 2 and j % 4 == 1:
        return 'v'
    return None
# out-projection psum->sbuf copy engine per block ('v' = DVE, 's' = Scalar)
COPY_ENG = {0: 'v', 1: 'v', 2: 's', 3: 's'}


def build_program():
    import concourse.tile as tile
    from concourse import mybir, bacc

    F32 = mybir.dt.float32
    BF16 = mybir.dt.bfloat16
    FP8 = mybir.dt.float8e4
    DR = mybir.MatmulPerfMode.DoubleRow
    AF = mybir.ActivationFunctionType
    MUL = mybir.AluOpType.mult
    ADD = mybir.AluOpType.add

    nc = bacc.Bacc("TRN2", target_bir_lowering=False, debug=False,
                   num_devices=NCORES)

    # x and w arrive as compensated fp8 pairs: x ~ (x8 + dx8)/16,
    # w ~ (w8 + dw8)/256 -> every product term is 4096x the true value and
    # accumulates in one PSUM group; the psum->sbuf copy rescales by 1/4096.
    xq = nc.dram_tensor("xq", [2, D, S], FP8, kind="ExternalInput").ap()
    xk = nc.dram_tensor("xk", [2, D, S], FP8, kind="ExternalInput").ap()
    xv = nc.dram_tensor("xv", [2, D, S], FP8, kind="ExternalInput").ap()
    wq = nc.dram_tensor("wq", [2, D, CPG], FP8, kind="ExternalInput").ap()
    wk = nc.dram_tensor("wk", [2, D, CPG], FP8, kind="ExternalInput").ap()
    wv = nc.dram_tensor("wv", [2, D, CPG], FP8, kind="ExternalInput").ap()
    wo = nc.dram_tensor("wo", [CPG, D], BF16, kind="ExternalInput").ap()
    # bqk f32-as-bf16-bytes (cols 0:16) | maskw (16:144) | ident (144:272)
    cst = nc.dram_tensor("cst", [128, 272], BF16, kind="ExternalInput").ap()
    o = nc.dram_tensor("o", [D, S], F32, kind="ExternalOutput").ap()

    xq_r = xq.rearrange("e (o p) s -> p e o s", p=128)
    xk_r = xk.rearrange("e (o p) s -> p e o s", p=128)
    xv_r = xv.rearrange("e (o p) s -> p e o s", p=128)
    wq_r = wq.rearrange("e (o p) c -> p e o c", p=128)
    wk_r = wk.rearrange("e (o p) c -> p e o c", p=128)
    wv_r = wv.rearrange("e (o p) c -> p e o c", p=128)
    wo_r = wo.rearrange("(o p) c -> p o c", p=128)

    with tile.TileContext(nc) as tc:
        with (
            tc.tile_pool(name="wts", bufs=1) as wts,
            tc.tile_pool(name="kv", bufs=1) as kv,
            tc.tile_pool(name="xstr", bufs=2) as xstr,
            tc.tile_pool(name="egp", bufs=2) as egp,
            tc.tile_pool(name="ab", bufs=2) as abp,
            tc.tile_pool(name="sfp", bufs=3) as sfp,
            tc.tile_pool(name="ps_pg", bufs=2, space="PSUM") as ps_pg,
            tc.tile_pool(name="ps_pv", bufs=2, space="PSUM") as ps_pv,
            tc.tile_pool(name="ps_mm", bufs=2, space="PSUM") as ps_mm,
        ):
            # ---- constants (single DMA) ----
            cst_t = wts.tile([128, 272], BF16)
            nc.sync.dma_start(out=cst_t[:], in_=cst[:])
            bqk_t = cst_t[:, 0:16].bitcast(F32)
            maskw_t = cst_t[:, 16:144]
            ident_t = cst_t[:, 144:272]

            # ---- persistent weights/activations ----
            wq_t = wts.tile([128, 2, NDT, CPG], FP8)
            wk_t = wts.tile([128, 2, NDT, CPG], FP8)
            wv_t = wts.tile([128, 2, NDT, CPG], FP8)
            wo_t = wts.tile([128, NCT, D], BF16)
            # q/k kept fp8 in a DoubleRow-friendly layout: partition =
            # (head%4)*32 + dk%32, dim1 = head-group g2, dim2 = dk//32
            kT_t = kv.tile([128, 2, 2, S], FP8)
            vhx_t = kv.tile([128, NST, HPC, DK + 1], BF16)

            # compensated-fp8 term order: (stationary_e, moving_e)
            TERMS = ((0, 0), (0, 1), (1, 0))

            def qproj(blk, b2s=(0, 1, 2, 3), xq_t=None):
                sq0 = blk * SQB
                if xq_t is None:
                    xq_t = xstr.tile([128, 2, NDT, SQB], FP8, tag="xq",
                                     name="xq_t")
                    nc.sync.dma_start(out=xq_t[:],
                                      in_=xq_r[:, :, :, sq0:sq0 + SQB])
                    qproj.qT = xstr.tile([128, 2, 2, SQB], FP8, tag="qT",
                                         name="qT_t")
                qT_t = qproj.qT
                for b2 in b2s:
                    g2, kh = b2 // 2, b2 % 2
                    pq = ps_mm.tile([128, SQB], F32, tag="mm")
                    for ti, (we, xe) in enumerate(TERMS):
                        for p in range(4):
                            nc.tensor.matmul(
                                pq[:],
                                wq_t[:, we, 2 * p:2 * p + 2,
                                     b2 * 128:(b2 + 1) * 128],
                                xq_t[:, xe, 2 * p:2 * p + 2, :],
                                start=(ti == 0 and p == 0),
                                stop=(ti == 2 and p == 3), perf_mode=DR)
                    nc.vector.tensor_scalar(qT_t[:, g2, kh, :], pq[:],
                                            1.0 / 4096, bqk_t[:, b2:b2 + 1],
                                            MUL, ADD)
                return qT_t, xq_t

            def kproj(blk, b2s=(0, 1, 2, 3), xk_t=None):
                sq0 = blk * SQB
                if xk_t is None:
                    xk_t = xstr.tile([128, 2, NDT, SQB], FP8, tag="xk",
                                     name="xk_t")
                    nc.sync.dma_start(out=xk_t[:],
                                      in_=xk_r[:, :, :, sq0:sq0 + SQB])
                for b2 in b2s:
                    g2, kh = b2 // 2, b2 % 2
                    pk = ps_mm.tile([128, SQB], F32, tag="mm")
                    for ti, (we, xe) in enumerate(TERMS):
                        for p in range(4):
                            nc.tensor.matmul(
                                pk[:],
                                wk_t[:, we, 2 * p:2 * p + 2,
                                     b2 * 128:(b2 + 1) * 128],
                                xk_t[:, xe, 2 * p:2 * p + 2, :],
                                start=(ti == 0 and p == 0),
                                stop=(ti == 2 and p == 3), perf_mode=DR)
                    nc.vector.tensor_scalar(
                        kT_t[:, g2, kh, sq0:sq0 + SQB], pk[:],
                        1.0 / 4096, bqk_t[:, 4 + b2:5 + b2], MUL, ADD)
                return xk_t

            def vproj(st_lo, st_hi, chunk):
                if chunk is not None:
                    xv_t = xstr.tile([128, 2, NDT, SQB], FP8, tag="xv",
                                     name="xv_t")
                    vproj.xv = xv_t
                    nc.sync.dma_start(
                        out=xv_t[:],
                        in_=xv_r[:, :, :, chunk * SQB:(chunk + 1) * SQB])
                xv_t = vproj.xv
                for st in range(st_lo, st_hi):
                    s0 = (st % 4) * 128
                    pv = ps_mm.tile([128, SQB], F32, tag="mm")
                    for ti, (xe, we) in enumerate(TERMS):
                        for p in range(4):
                            nc.tensor.matmul(
                                pv[:],
                                xv_t[:, xe, 2 * p:2 * p + 2, s0:s0 + 128],
                                wv_t[:, we, 2 * p:2 * p + 2, :],
                                start=(ti == 0 and p == 0),
                                stop=(ti == 2 and p == 3), perf_mode=DR)
                    nc.vector.tensor_scalar_mul(
                        vhx_t[:, st, :, 0:DK],
                        pv.rearrange("p (h d) -> p h d", h=HPC), 1.0 / 4096)

            # bf16-bits Schraudolph exp for DVE offload:
            # int16(score * 0.125*log2(e)*128 + (127*128 - 5.09)) ~ bf16 bits
            # of exp(score/8); max rel err ~3.3%, error-neutral end to end
            # when restricted to blk3 odd k-tiles (measured).
            SCHR_A = 0.125 * float(np.log2(np.e)) * 128.0
            SCHR_B = 127.0 * 128.0 - 5.09
            I16 = mybir.dt.int16

            def scores_exp(blk, t, qT_t):
                nsk = 4 * (blk + 1)
                eg = egp.tile([128, NST, 2, SQB], BF16, tag="eg")
                for j in range(nsk):
                    w0 = max(0, (j - 4 * blk) * 128)
                    pg = ps_pg.tile([128, 2, SQB], F32, tag="pg")
                    for hp in range(2):
                        h = 2 * t + hp
                        g2, h4 = h // 4, h % 4
                        nc.tensor.matmul(
                            pg[:, hp, w0:SQB],
                            kT_t[h4 * 32:h4 * 32 + 32, g2, :,
                                 j * 128:(j + 1) * 128],
                            qT_t[h4 * 32:h4 * 32 + 32, g2, :, w0:SQB],
                            start=True, stop=True, perf_mode=DR,
                            tile_position=(h4 * 32, 0))
                    se = SCHR_SEL(blk, j)
                    if se:
                        eng = nc.vector if se == 'v' else nc.gpsimd
                        eng.tensor_scalar(
                            eg[:, j, :, w0:SQB].bitcast(I16),
                            pg[:, :, w0:SQB], SCHR_A, SCHR_B, MUL, ADD)
                    else:
                        nc.scalar.activation(eg[:, j, :, w0:SQB],
                                             pg[:, :, w0:SQB],
                                             AF.Exp, bias=0.0, scale=0.125)
                    if w0 > 0 or j == 4 * blk:
                        nc.vector.tensor_tensor(
                            eg[:, j, :, w0:w0 + 128],
                            eg[:, j, :, w0:w0 + 128],
                            maskw_t.unsqueeze(1).to_broadcast((128, 2, 128)),
                            MUL)
                return eg

            def attnv_norm(blk, t, attn_sb, eg):
                for hp in range(2):
                    h = 2 * t + hp
                    pv = ps_pv.tile([128, 4, DK + 1], F32, tag="pv")
                    for sub in range(4):
                        ig = 4 * blk + sub
                        for j in range(ig + 1):
                            nc.tensor.matmul(
                                pv[:, sub, :],
                                eg[:, j, hp, sub * 128:sub * 128 + 128],
                                vhx_t[:, j, h, :],
                                start=(j == 0), stop=(j == ig))
                    recip = abp.tile([128, 4], F32, tag="recip")
                    nc.vector.reciprocal(recip[:], pv[:, :, DK])
                    nc.vector.tensor_tensor(
                        attn_sb[:, :, h * DK:(h + 1) * DK],
                        pv[:, :, 0:DK],
                        recip.unsqueeze(-1).to_broadcast((128, 4, DK)),
                        MUL)

            def transpose_out(blk, attn_sb):
                outT_sb = abp.tile([128, NCT, SQB], BF16, tag="outT")
                for half in range(2):
                    ptr = ps_mm.tile([128, 8, 128], BF16, tag="mm")
                    for i in range(2):
                        sub = half * 2 + i
                        for ct in range(NCT):
                            nc.tensor.transpose(
                                ptr[:, i * 4 + ct, :],
                                attn_sb[:, sub, ct * 128:(ct + 1) * 128],
                                ident_t[:])
                    dst = outT_sb.rearrange("p c (u q) -> p u c q", u=4)
                    nc.vector.tensor_copy(
                        dst[:, half * 2:half * 2 + 2, :, :],
                        ptr.rearrange("p (i c) q -> p i c q", i=2))
                return outT_sb

            def outproj(blk, outT_sb):
                sq0 = blk * SQB
                for dt_i in range(8):
                    pf = ps_mm.tile([128, SQB], F32, tag="mm")
                    for ct in range(NCT):
                        nc.tensor.matmul(
                            pf[:],
                            wo_t[:, ct, dt_i * 128:(dt_i + 1) * 128],
                            outT_sb[:, ct, :],
                            start=(ct == 0), stop=(ct == NCT - 1))
                    sf = sfp.tile([128, SQB], F32, tag="sf")
                    if COPY_ENG[blk] == 's':
                        nc.scalar.copy(sf[:], pf[:])
                    else:
                        nc.vector.tensor_copy(sf[:], pf[:])
                    nc.sync.dma_start(
                        out=o[dt_i * 128:(dt_i + 1) * 128, sq0:sq0 + SQB],
                        in_=sf[:])

            # ================= schedule =================
            # wq/wk arrive as half-DMAs so the first b2 blocks start early;
            # blk0's projections are interleaved with its first score groups
            # to get the Scalar engine (exp) running as soon as possible.
            nc.sync.dma_start(out=wq_t[:, :, :, 0:256],
                              in_=wq_r[:, :, :, 0:256])
            qT, xq0 = qproj(0, (0, 1))
            nc.sync.dma_start(out=wk_t[:, :, :, 0:256],
                              in_=wk_r[:, :, :, 0:256])
            xk0 = kproj(0, (0, 1))
            nc.sync.dma_start(out=wq_t[:, :, :, 256:512],
                              in_=wq_r[:, :, :, 256:512])
            nc.sync.dma_start(out=wk_t[:, :, :, 256:512],
                              in_=wk_r[:, :, :, 256:512])
            nc.sync.dma_start(out=wv_t[:], in_=wv_r[:, :, :, :])
            nc.vector.memset(vhx_t[:, :, :, DK], 1.0)
            nc.sync.dma_start(out=wo_t[:], in_=wo_r[:])

            # Flat depth-2 pipeline over groups g = 4*blk + t; the next
            # block's first score groups are prefetched into the current
            # block's attnv stream so exp never starves at block boundaries.
            qTs = {0: qT}
            egs = {}
            attn_sbs = {0: abp.tile([128, 4, CPG], BF16, tag="attn",
                                    name="attn_sb0")}
            outTs = {}

            def sc(g):
                blk, t = divmod(g, 4)
                egs[g] = scores_exp(blk, t, qTs[blk])

            def av(g):
                blk, t = divmod(g, 4)
                attnv_norm(blk, t, attn_sbs[blk], egs[g])

            def transp(blk):
                outTs[blk] = transpose_out(blk, attn_sbs[blk])
                if blk + 1 < NB:
                    attn_sbs[blk + 1] = abp.tile(
                        [128, 4, CPG], BF16, tag="attn",
                        name=f"attn_sb{blk + 1}")

            sc(0)
            sc(1)
            # --- blk0 ---
            qproj(0, (2, 3), xq0)
            kproj(0, (2, 3), xk0)
            vproj(0, 4, 0)
            av(0); sc(2)
            vproj(4, 8, 1)
            av(1); sc(3)
            kproj(1)
            av(2)
            qTs[1], _ = qproj(1)
            sc(4)
            av(3); transp(0); sc(5)
            # --- blk1 ---
            av(4); sc(6)
            outproj(0, outTs[0])
            av(5); sc(7)
            kproj(2)
            av(6)
            qTs[2], _ = qproj(2)
            sc(8)
            av(7); transp(1); sc(9)
            # --- blk2 ---
            vproj(8, 12, 2)
            av(8); sc(10)
            outproj(1, outTs[1])
            av(9); sc(11)
            kproj(3)
            av(10)
            qTs[3], _ = qproj(3)
            sc(12)
            av(11); transp(2); sc(13)
            # --- blk3 ---
            vproj(12, 16, 3)
            av(12); sc(14)
            outproj(2, outTs[2])
            av(13); sc(15)
            av(14)
            av(15); transp(3)
            outproj(3, outTs[3])

    nc.compile()
    return nc


def _get_program():
    global _PROGRAM
    if _PROGRAM is None:
        _PROGRAM = build_program()
    return _PROGRAM


def _make_maskw():
    p = np.arange(128, dtype=np.int64)[:, None]
    f = np.arange(128, dtype=np.int64)[None, :]
    return (f >= p).astype(np.float32)


def _qk_perm():
    # column order for the DoubleRow-friendly q/k layout:
    # block b2=(g2, dk_half): partition = (head%4)*32 + dk%32
    perm = np.empty(CPG, np.int64)
    for g2 in range(2):
        for kh in range(2):
            for h4 in range(4):
                base = (g2 * 2 + kh) * 128 + h4 * 32
                src = (4 * g2 + h4) * 64 + kh * 32
                perm[base:base + 32] = np.arange(src, src + 32)
    return perm


def _comp8(a):
    """Scaled f32 array -> stacked (high, delta) fp8 pair, a ~ h + d."""
    import ml_dtypes
    E4 = ml_dtypes.float8_e4m3
    h = a.astype(E4)
    d = (a - h.astype(np.float32)).astype(E4)
    return np.ascontiguousarray(np.stack([h, d]))


def make_in_maps(q, k, v, Wq, bq, Wk, bk, Wv, bv, Wo):
    import ml_dtypes
    BF = ml_dtypes.bfloat16
    wqT, wkT, wvT, woT = Wq.T, Wk.T, Wv.T, Wo.T
    mw = _make_maskw()
    perm = _qk_perm()
    xq8 = [_comp8(16.0 * q[b].T) for b in range(B)]
    xk8 = [_comp8(16.0 * k[b].T) for b in range(B)]
    xv8 = [_comp8(16.0 * v[b].T) for b in range(B)]
    in_maps = []
    for core in range(NCORES):
        b, g = core // 2, core % 2
        cs = slice(g * CPG, (g + 1) * CPG)
        bq_p = bq[cs][perm]
        bk_p = bk[cs][perm]
        cst_host = np.zeros((128, 272), BF)
        bqk_f32 = cst_host[:, 0:16].view(np.float32)
        for b2 in range(4):
            bqk_f32[:, b2] = bq_p[b2 * 128:(b2 + 1) * 128]
            bqk_f32[:, 4 + b2] = bk_p[b2 * 128:(b2 + 1) * 128]
        cst_host[:, 16:144] = mw.astype(BF)
        cst_host[:, 144:272] = np.eye(128, dtype=np.float32).astype(BF)
        in_maps.append(dict(
            xq=xq8[b], xk=xk8[b], xv=xv8[b],
            wq=_comp8(256.0 * wqT[:, cs][:, perm]),
            wk=_comp8(256.0 * wkT[:, cs][:, perm]),
            wv=_comp8(256.0 * wvT[:, cs]),
            wo=np.ascontiguousarray(woT[cs, :].astype(BF)),
            cst=cst_host,
        ))
    return in_maps


def assemble_output(results, bv, Wo, bo):
    hb = (bo + bv @ Wo.T).astype(np.float32)
    out = np.empty((B, S, D), np.float32)
    for b in range(B):
        acc = results[2 * b]["o"] + results[2 * b + 1]["o"]  # [D, S]
        out[b] = acc.T + hb[None, :]
    return out


def _numpy_fallback(q, k, v, mask, Wq, bq, Wk, bk, Wv, bv, Wo, bo):
    def split_heads(x):
        return x.reshape(B, S, H, DK).transpose(0, 2, 1, 3)

    qh = split_heads(q @ Wq.T + bq)
    kh = split_heads(k @ Wk.T + bk)
    vh = split_heads(v @ Wv.T + bv)
    out = np.empty((B, H, S, DK), np.float32)
    m = np.broadcast_to(np.asarray(mask).reshape(-1, S, S)[-1], (S, S))
    for b in range(B):
        for h in range(H):
            s = (qh[b, h] @ kh[b, h].T) / np.float32(np.sqrt(DK))
            s = np.where(m == 0, np.float32(-1e9), s)
            s = s - s.max(axis=-1, keepdims=True)
            e = np.exp(s)
            a = e / e.sum(axis=-1, keepdims=True)
            out[b, h] = a @ vh[b, h]
    out = out.transpose(0, 2, 1, 3).reshape(B, S, D)
    return out @ Wo.T + bo


def kernel(q, k, v, mask, Wq, bq, Wk, bk, Wv, bv, Wo, bo):
    from concourse.bass_utils import run_bass_kernel_spmd

    q = np.ascontiguousarray(np.asarray(q), dtype=np.float32)
    k = np.ascontiguousarray(np.asarray(k), dtype=np.float32)
    v = np.ascontiguousarray(np.asarray(v), dtype=np.float32)
    Wq, Wk, Wv, Wo = (np.asarray(w, dtype=np.float32) for w in (Wq, Wk, Wv, Wo))
    bq, bk_, bv_, bo = (np.asarray(x, dtype=np.float32) for x in (bq, bk, bv, bo))

    mask_2d = np.asarray(mask).reshape(S, S)
    causal = bool(np.array_equal(mask_2d != 0, np.tril(np.ones((S, S), bool))))
    if not causal:
        return _numpy_fallback(q, k, v, mask, Wq, bq, Wk, bk_, Wv, bv_, Wo, bo)

    nc = _get_program()
    in_maps = make_in_maps(q, k, v, Wq, bq, Wk, bk_, Wv, bv_, Wo)
    res = run_bass_kernel_spmd(nc, in_maps, list(range(NCORES))).results
    return assemble_output(res, bv_, Wo, bo)


if __name__ == "__main__":
    nc = build_program()
    print("program built + compiled OK")


# revision 78
# speedup vs baseline: 1.0049x; 1.0049x over previous
"""Multi-head causal attention (B=4, S=2048, D=1024, H=16) on 8 trn2 NeuronCores.

Sharding: core = (batch b, head-group g), g in 0..1; each core computes heads
g*8..g*8+7 of batch b end-to-end plus its partial output projection; host sums
the two partials per batch and adds (bo + bv @ Wo.T) (the v-bias folds into
the host-side bias since softmax weights sum to 1).

Per-core dataflow, shaped for a PE whose matmul time is
(output free size) x (cycles/row of the moving dtype):
  QKV proj   compensated fp8 DoubleRow: x ~ (x8+dx8)/16, w ~ (w8+dw8)/256
             (host-split pairs), 3 product terms at a uniform 4096x PSUM
             scale -> 0.75x the bf16 row count at ~bf16 accuracy; the
             psum->sbuf copy rescales by 1/4096 and adds the q/k biases.
  scores     q/k stored fp8e4 in a DoubleRow layout (partition =
             (head%4)*32 + dk%32, k-tile dim = dk//32): one 0.5-cyc/row
             matmul per (head, k-tile), causal-tight sq windows.
  exp        Scalar-engine Exp into bf16 eg tiles; a measured-safe subset
             of late-block tiles computes exp on DVE instead via an int16
             Schraudolph (bf16-bits magic constant), balancing engines.
  attn@v     FLIPPED: stationary eg [sk, sq-128], moving vhx [sk, 65]
             (bf16) -> 65 rows/matmul; softmax denominator rides in column
             64 (ones column of vhx); causal masks only touch the 16
             true-diagonal 128x128 triangles.
  normalize  per-sq-partition reciprocal [128,4] + one broadcast multiply.
  transpose  PE transpose attn_out [sq,c]->[c,sq] against a bf16 identity.
  out proj   stationary wo bf16, moving outT bf16; last block's psum
             copies run on the (then idle) Scalar engine.

Measured on the harness input: ~200us/core (TimelineSim), rel err 7.8e-3
vs the fp32 reference (gate 2e-2).
"""

import sys

sys.path.insert(0, "/opt/trn_rl_repo")

import numpy as np

B, S, D, H, DK = 4, 2048, 1024, 16, 64
NCORES = 8
CPG = 512          # channels per core (8 heads)
HPC = 8            # heads per core
NB = 4             # sq blocks of 512
SQB = 512
NDT = D // 128     # 8 d-tiles
NCT = CPG // 128   # 4 c-tiles per core (= head pairs)
NST = S // 128     # 16 s-tiles

_PROGRAM = None

# exp tiles offloaded from Scalar via bf16 Schraudolph; returns None (keep
# Scalar Exp), 'v' (DVE) or 'p' (gpsimd). Subset must have measured-good
# end-to-end error.
def SCHR_SEL(blk, j):
    if blk == 3 and j % 2 == 1:
        return 'v'
    if blk == 2 and j % 4 == 1:
        return 'v'
    return None
# out-projection psum->sbuf copy engine per block ('v' = DVE, 's' = Scalar)
COPY_ENG = {0: 'v', 1: 'v', 2: 's', 3: 's'}


def build_program():
    import concourse.tile as tile
    from concourse import mybir, bacc

    F32 = mybir.dt.float32
    BF16 = mybir.dt.bfloat16
    FP8 = mybir.dt.float8e4
    DR = mybir.MatmulPerfMode.DoubleRow
    AF = mybir.ActivationFunctionType
    MUL = mybir.AluOpType.mult
    ADD = mybir.AluOpType.add

    nc = bacc.Bacc("TRN2", target_bir_lowering=False, debug=False,
                   num_devices=NCORES)

    # x and w arrive as compensated fp8 pairs: x ~ (x8 + dx8)/16,
    # w ~ (w8 + dw8)/256 -> every product term is 4096x the true value and
    # accumulates in one PSUM group; the psum->sbuf copy rescales by 1/4096.
    xq = nc.dram_tensor("xq", [2, D, S], FP8, kind="ExternalInput").ap()
    xk = nc.dram_tensor("xk", [2, D, S], FP8, kind="ExternalInput").ap()
    xv = nc.dram_tensor("xv", [2, D, S], FP8, kind="ExternalInput").ap()
    wq = nc.dram_tensor("wq", [2, D, CPG], FP8, kind="ExternalInput").ap()
    wk = nc.dram_tensor("wk", [2, D, CPG], FP8, kind="ExternalInput").ap()
    wv = nc.dram_tensor("wv", [2, D, CPG], FP8, kind="ExternalInput").ap()
    wo = nc.dram_tensor("wo", [CPG, D], BF16, kind="ExternalInput").ap()
    # bqk f32-as-bf16-bytes (cols 0:16) | maskw (16:144) | ident (144:272)
    cst = nc.dram_tensor("cst", [128, 272], BF16, kind="ExternalInput").ap()
    o = nc.dram_tensor("o", [D, S], F32, kind="ExternalOutput").ap()

    xq_r = xq.rearrange("e (o p) s -> p e o s", p=128)
    xk_r = xk.rearrange("e (o p) s -> p e o s", p=128)
    xv_r = xv.rearrange("e (o p) s -> p e o s", p=128)
    wq_r = wq.rearrange("e (o p) c -> p e o c", p=128)
    wk_r = wk.rearrange("e (o p) c -> p e o c", p=128)
    wv_r = wv.rearrange("e (o p) c -> p e o c", p=128)
    wo_r = wo.rearrange("(o p) c -> p o c", p=128)

    with tile.TileContext(nc) as tc:
        with (
            tc.tile_pool(name="wts", bufs=1) as wts,
            tc.tile_pool(name="kv", bufs=1) as kv,
            tc.tile_pool(name="xstr", bufs=2) as xstr,
            tc.tile_pool(name="egp", bufs=2) as egp,
            tc.tile_pool(name="ab", bufs=2) as abp,
            tc.tile_pool(name="sfp", bufs=3) as sfp,
            tc.tile_pool(name="ps_pg", bufs=2, space="PSUM") as ps_pg,
            tc.tile_pool(name="ps_pv", bufs=2, space="PSUM") as ps_pv,
            tc.tile_pool(name="ps_mm", bufs=2, space="PSUM") as ps_mm,
        ):
            # ---- constants (single DMA) ----
            cst_t = wts.tile([128, 272], BF16)
            nc.sync.dma_start(out=cst_t[:], in_=cst[:])
            bqk_t = cst_t[:, 0:16].bitcast(F32)
            maskw_t = cst_t[:, 16:144]
            ident_t = cst_t[:, 144:272]

            # ---- persistent weights/activations ----
            wq_t = wts.tile([128, 2, NDT, CPG], FP8)
            wk_t = wts.tile([128, 2, NDT, CPG], FP8)
            wv_t = wts.tile([128, 2, NDT, CPG], FP8)
            wo_t = wts.tile([128, NCT, D], BF16)
            # q/k kept fp8 in a DoubleRow-friendly layout: partition =
            # (head%4)*32 + dk%32, dim1 = head-group g2, dim2 = dk//32
            kT_t = kv.tile([128, 2, 2, S], FP8)
            vhx_t = kv.tile([128, NST, HPC, DK + 1], BF16)

            # compensated-fp8 term order: (stationary_e, moving_e)
            TERMS = ((0, 0), (0, 1), (1, 0))

            def qproj(blk, b2s=(0, 1, 2, 3), xq_t=None):
                sq0 = blk * SQB
                if xq_t is None:
                    xq_t = xstr.tile([128, 2, NDT, SQB], FP8, tag="xq",
                                     name="xq_t")
                    nc.sync.dma_start(out=xq_t[:],
                                      in_=xq_r[:, :, :, sq0:sq0 + SQB])
                    qproj.qT = xstr.tile([128, 2, 2, SQB], FP8, tag="qT",
                                         name="qT_t")
                qT_t = qproj.qT
                for b2 in b2s:
                    g2, kh = b2 // 2, b2 % 2
                    pq = ps_mm.tile([128, SQB], F32, tag="mm")
                    for ti, (we, xe) in enumerate(TERMS):
                        for p in range(4):
                            nc.tensor.matmul(
                                pq[:],
                                wq_t[:, we, 2 * p:2 * p + 2,
                                     b2 * 128:(b2 + 1) * 128],
                                xq_t[:, xe, 2 * p:2 * p + 2, :],
                                start=(ti == 0 and p == 0),
                                stop=(ti == 2 and p == 3), perf_mode=DR)
                    nc.vector.tensor_scalar(qT_t[:, g2, kh, :], pq[:],
                                            1.0 / 4096, bqk_t[:, b2:b2 + 1],
                                            MUL, ADD)
                return qT_t, xq_t

            def kproj(blk, b2s=(0, 1, 2, 3), xk_t=None):
                sq0 = blk * SQB
                if xk_t is None:
                    xk_t = xstr.tile([128, 2, NDT, SQB], FP8, tag="xk",
                                     name="xk_t")
                    nc.sync.dma_start(out=xk_t[:],
                                      in_=xk_r[:, :, :, sq0:sq0 + SQB])
                for b2 in b2s:
                    g2, kh = b2 // 2, b2 % 2
                    pk = ps_mm.tile([128, SQB], F32, tag="mm")
                    for ti, (we, xe) in enumerate(TERMS):
                        for p in range(4):
                            nc.tensor.matmul(
                                pk[:],
                                wk_t[:, we, 2 * p:2 * p + 2,
                                     b2 * 128:(b2 + 1) * 128],
                                xk_t[:, xe, 2 * p:2 * p + 2, :],
                                start=(ti == 0 and p == 0),
                                stop=(ti == 2 and p == 3), perf_mode=DR)
                    nc.vector.tensor_scalar(
                        kT_t[:, g2, kh, sq0:sq0 + SQB], pk[:],
                        1.0 / 4096, bqk_t[:, 4 + b2:5 + b2], MUL, ADD)
                return xk_t

            def vproj(st_lo, st_hi, chunk):
                if chunk is not None:
                    xv_t = xstr.tile([128, 2, NDT, SQB], FP8, tag="xv",
                                     name="xv_t")
                    vproj.xv = xv_t
                    nc.sync.dma_start(
                        out=xv_t[:],
                        in_=xv_r[:, :, :, chunk * SQB:(chunk + 1) * SQB])
                xv_t = vproj.xv
                for st in range(st_lo, st_hi):
                    s0 = (st % 4) * 128
                    pv = ps_mm.tile([128, SQB], F32, tag="mm")
                    for ti, (xe, we) in enumerate(TERMS):
                        for p in range(4):
                            nc.tensor.matmul(
                                pv[:],
                                xv_t[:, xe, 2 * p:2 * p + 2, s0:s0 + 128],
                                wv_t[:, we, 2 * p:2 * p + 2, :],
                                start=(ti == 0 and p == 0),
                                stop=(ti == 2 and p == 3), perf_mode=DR)
                    nc.vector.tensor_scalar_mul(
                        vhx_t[:, st, :, 0:DK],
                        pv.rearrange("p (h d) -> p h d", h=HPC), 1.0 / 4096)

            # bf16-bits Schraudolph exp for DVE offload:
            # int16(score * 0.125*log2(e)*128 + (127*128 - 5.09)) ~ bf16 bits
            # of exp(score/8); max rel err ~3.3%, error-neutral end to end
            # when restricted to blk3 odd k-tiles (measured).
            SCHR_A = 0.125 * float(np.log2(np.e)) * 128.0
            SCHR_B = 127.0 * 128.0 - 5.09
            I16 = mybir.dt.int16

            def scores_exp(blk, t, qT_t):
                nsk = 4 * (blk + 1)
                eg = egp.tile([128, NST, 2, SQB], BF16, tag="eg")
                for j in range(nsk):
                    w0 = max(0, (j - 4 * blk) * 128)
                    pg = ps_pg.tile([128, 2, SQB], F32, tag="pg")
                    for hp in range(2):
                        h = 2 * t + hp
                        g2, h4 = h // 4, h % 4
                        nc.tensor.matmul(
                            pg[:, hp, w0:SQB],
                            kT_t[h4 * 32:h4 * 32 + 32, g2, :,
                                 j * 128:(j + 1) * 128],
                            qT_t[h4 * 32:h4 * 32 + 32, g2, :, w0:SQB],
                            start=True, stop=True, perf_mode=DR,
                            tile_position=(h4 * 32, 0))
                    se = SCHR_SEL(blk, j)
                    if se:
                        eng = nc.vector if se == 'v' else nc.gpsimd
                        eng.tensor_scalar(
                            eg[:, j, :, w0:SQB].bitcast(I16),
                            pg[:, :, w0:SQB], SCHR_A, SCHR_B, MUL, ADD)
                    else:
                        nc.scalar.activation(eg[:, j, :, w0:SQB],
                                             pg[:, :, w0:SQB],
                                             AF.Exp, bias=0.0, scale=0.125)
                    if w0 > 0 or j == 4 * blk:
                        nc.vector.tensor_tensor(
                            eg[:, j, :, w0:w0 + 128],
                            eg[:, j, :, w0:w0 + 128],
                            maskw_t.unsqueeze(1).to_broadcast((128, 2, 128)),
                            MUL)
                return eg

            def attnv_norm(blk, t, attn_sb, eg):
                for hp in range(2):
                    h = 2 * t + hp
                    pv = ps_pv.tile([128, 4, DK + 1], F32, tag="pv")
                    for sub in range(4):
                        ig = 4 * blk + sub
                        for j in range(ig + 1):
                            nc.tensor.matmul(
                                pv[:, sub, :],
                                eg[:, j, hp, sub * 128:sub * 128 + 128],
                                vhx_t[:, j, h, :],
                                start=(j == 0), stop=(j == ig))
                    recip = abp.tile([128, 4], F32, tag="recip")
                    nc.vector.reciprocal(recip[:], pv[:, :, DK])
                    nc.vector.tensor_tensor(
                        attn_sb[:, :, h * DK:(h + 1) * DK],
                        pv[:, :, 0:DK],
                        recip.unsqueeze(-1).to_broadcast((128, 4, DK)),
                        MUL)

            def transpose_out(blk, attn_sb):
                outT_sb = abp.tile([128, NCT, SQB], BF16, tag="outT")
                for half in range(2):
                    ptr = ps_mm.tile([128, 8, 128], BF16, tag="mm")
                    for i in range(2):
                        sub = half * 2 + i
                        for ct in range(NCT):
                            nc.tensor.transpose(
                                ptr[:, i * 4 + ct, :],
                                attn_sb[:, sub, ct * 128:(ct + 1) * 128],
                                ident_t[:])
                    dst = outT_sb.rearrange("p c (u q) -> p u c q", u=4)
                    nc.vector.tensor_copy(
                        dst[:, half * 2:half * 2 + 2, :, :],
                        ptr.rearrange("p (i c) q -> p i c q", i=2))
                return outT_sb

            def transpose_part(attn_sb, outT_sb, cts):
                # transpose a subset of head-pair columns (all 4 sq-subs);
                # lets the last block's ct 0..2 run before its final norm
                dst = outT_sb.rearrange("p c (u q) -> p u c q", u=4)
                nct = len(cts)
                for half in range(2):
                    ptr = ps_mm.tile([128, 2 * nct, 128], BF16, tag="mm",
                                     name="ptr")
                    for i in range(2):
                        sub = half * 2 + i
                        for ci, ct in enumerate(cts):
                            nc.tensor.transpose(
                                ptr[:, i * nct + ci, :],
                                attn_sb[:, sub, ct * 128:(ct + 1) * 128],
                                ident_t[:])
                    nc.vector.tensor_copy(
                        dst[:, half * 2:half * 2 + 2, cts[0]:cts[0] + nct,
                            :],
                        ptr.rearrange("p (i c) q -> p i c q", i=2))

            def outproj(blk, outT_sb):
                sq0 = blk * SQB
                for dt_i in range(8):
                    pf = ps_mm.tile([128, SQB], F32, tag="mm")
                    for ct in range(NCT):
                        nc.tensor.matmul(
                            pf[:],
                            wo_t[:, ct, dt_i * 128:(dt_i + 1) * 128],
                            outT_sb[:, ct, :],
                            start=(ct == 0), stop=(ct == NCT - 1))
                    sf = sfp.tile([128, SQB], F32, tag="sf")
                    if COPY_ENG[blk] == 's':
                        nc.scalar.copy(sf[:], pf[:])
                    else:
                        nc.vector.tensor_copy(sf[:], pf[:])
                    nc.sync.dma_start(
                        out=o[dt_i * 128:(dt_i + 1) * 128, sq0:sq0 + SQB],
                        in_=sf[:])

            # ================= schedule =================
            # wq/wk arrive as half-DMAs so the first b2 blocks start early;
            # blk0's projections are interleaved with its first score groups
            # to get the Scalar engine (exp) running as soon as possible.
            nc.sync.dma_start(out=wq_t[:, :, :, 0:256],
                              in_=wq_r[:, :, :, 0:256])
            qT, xq0 = qproj(0, (0, 1))
            nc.sync.dma_start(out=wk_t[:, :, :, 0:256],
                              in_=wk_r[:, :, :, 0:256])
            xk0 = kproj(0, (0, 1))
            nc.sync.dma_start(out=wq_t[:, :, :, 256:512],
                              in_=wq_r[:, :, :, 256:512])
            nc.sync.dma_start(out=wk_t[:, :, :, 256:512],
                              in_=wk_r[:, :, :, 256:512])
            nc.sync.dma_start(out=wv_t[:], in_=wv_r[:, :, :, :])
            nc.vector.memset(vhx_t[:, :, :, DK], 1.0)
            nc.sync.dma_start(out=wo_t[:], in_=wo_r[:])

            # Flat depth-2 pipeline over groups g = 4*blk + t; the next
            # block's first score groups are prefetched into the current
            # block's attnv stream so exp never starves at block boundaries.
            qTs = {0: qT}
            egs = {}
            attn_sbs = {0: abp.tile([128, 4, CPG], BF16, tag="attn",
                                    name="attn_sb0")}
            outTs = {}

            def sc(g):
                blk, t = divmod(g, 4)
                egs[g] = scores_exp(blk, t, qTs[blk])

            def av(g):
                blk, t = divmod(g, 4)
                attnv_norm(blk, t, attn_sbs[blk], egs[g])

            def transp(blk):
                outTs[blk] = transpose_out(blk, attn_sbs[blk])
                if blk + 1 < NB:
                    attn_sbs[blk + 1] = abp.tile(
                        [128, 4, CPG], BF16, tag="attn",
                        name=f"attn_sb{blk + 1}")

            sc(0)
            sc(1)
            # --- blk0 ---
            qproj(0, (2, 3), xq0)
            kproj(0, (2, 3), xk0)
            vproj(0, 4, 0)
            av(0); sc(2)
            vproj(4, 8, 1)
            av(1); sc(3)
            kproj(1)
            av(2)
            qTs[1], _ = qproj(1)
            sc(4)
            av(3); transp(0); sc(5)
            # --- blk1 ---
            av(4); sc(6)
            outproj(0, outTs[0])
            av(5); sc(7)
            kproj(2)
            av(6)
            qTs[2], _ = qproj(2)
            sc(8)
            av(7); transp(1); sc(9)
            # --- blk2 ---
            vproj(8, 12, 2)
            av(8); sc(10)
            outproj(1, outTs[1])
            av(9); sc(11)
            kproj(3)
            av(10)
            qTs[3], _ = qproj(3)
            sc(12)
            av(11); transp(2); sc(13)
            # --- blk3 ---
            vproj(12, 16, 3)
            av(12); sc(14)
            outproj(2, outTs[2])
            av(13); sc(15)
            av(14)
            outT3 = abp.tile([128, NCT, SQB], BF16, tag="outT",
                             name="outT3")
            transpose_part(attn_sbs[3], outT3, (0, 1, 2))
            av(15)
            transpose_part(attn_sbs[3], outT3, (3,))
            outproj(3, outT3)

    nc.compile()
    return nc


def _get_program():
    global _PROGRAM
    if _PROGRAM is None:
        _PROGRAM = build_program()
    return _PROGRAM


def _make_maskw():
    p = np.arange(128, dtype=np.int64)[:, None]
    f = np.arange(128, dtype=np.int64)[None, :]
    return (f >= p).astype(np.float32)


def _qk_perm():
    # column order for the DoubleRow-friendly q/k layout:
    # block b2=(g2, dk_half): partition = (head%4)*32 + dk%32
    perm = np.empty(CPG, np.int64)
    for g2 in range(2):
        for kh in range(2):
            for h4 in range(4):
                base = (g2 * 2 + kh) * 128 + h4 * 32
                src = (4 * g2 + h4) * 64 + kh * 32
                perm[base:base + 32] = np.arange(src, src + 32)
    return perm


def _comp8(a):
    """Scaled f32 array -> stacked (high, delta) fp8 pair, a ~ h + d."""
    import ml_dtypes
    E4 = ml_dtypes.float8_e4m3
    h = a.astype(E4)
    d = (a - h.astype(np.float32)).astype(E4)
    return np.ascontiguousarray(np.stack([h, d]))


def make_in_maps(q, k, v, Wq, bq, Wk, bk, Wv, bv, Wo):
    import ml_dtypes
    BF = ml_dtypes.bfloat16
    wqT, wkT, wvT, woT = Wq.T, Wk.T, Wv.T, Wo.T
    mw = _make_maskw()
    perm = _qk_perm()
    xq8 = [_comp8(16.0 * q[b].T) for b in range(B)]
    xk8 = [_comp8(16.0 * k[b].T) for b in range(B)]
    xv8 = [_comp8(16.0 * v[b].T) for b in range(B)]
    in_maps = []
    for core in range(NCORES):
        b, g = core // 2, core % 2
        cs = slice(g * CPG, (g + 1) * CPG)
        bq_p = bq[cs][perm]
        bk_p = bk[cs][perm]
        cst_host = np.zeros((128, 272), BF)
        bqk_f32 = cst_host[:, 0:16].view(np.float32)
        for b2 in range(4):
            bqk_f32[:, b2] = bq_p[b2 * 128:(b2 + 1) * 128]
            bqk_f32[:, 4 + b2] = bk_p[b2 * 128:(b2 + 1) * 128]
        cst_host[:, 16:144] = mw.astype(BF)
        cst_host[:, 144:272] = np.eye(128, dtype=np.float32).astype(BF)
        in_maps.append(dict(
            xq=xq8[b], xk=xk8[b], xv=xv8[b],
            wq=_comp8(256.0 * wqT[:, cs][:, perm]),
            wk=_comp8(256.0 * wkT[:, cs][:, perm]),
            wv=_comp8(256.0 * wvT[:, cs]),
            wo=np.ascontiguousarray(woT[cs, :].astype(BF)),
            cst=cst_host,
        ))
    return in_maps


def assemble_output(results, bv, Wo, bo):
    hb = (bo + bv @ Wo.T).astype(np.float32)
    out = np.empty((B, S, D), np.float32)
    for b in range(B):
        acc = results[2 * b]["o"] + results[2 * b + 1]["o"]  # [D, S]
        out[b] = acc.T + hb[None, :]
    return out


def _numpy_fallback(q, k, v, mask, Wq, bq, Wk, bk, Wv, bv, Wo, bo):
    def split_heads(x):
        return x.reshape(B, S, H, DK).transpose(0, 2, 1, 3)

    qh = split_heads(q @ Wq.T + bq)
    kh = split_heads(k @ Wk.T + bk)
    vh = split_heads(v @ Wv.T + bv)
    out = np.empty((B, H, S, DK), np.float32)
    m = np.broadcast_to(np.asarray(mask).reshape(-1, S, S)[-1], (S, S))
    for b in range(B):
        for h in range(H):
            s = (qh[b, h] @ kh[b, h].T) / np.float32(np.sqrt(DK))
            s = np.where(m == 0, np.float32(-1e9), s)
            s = s - s.max(axis=-1, keepdims=True)
            e = np.exp(s)
            a = e / e.sum(axis=-1, keepdims=True)
            out[b, h] = a @ vh[b, h]
    out = out.transpose(0, 2, 1, 3).reshape(B, S, D)
    return out @ Wo.T + bo


def kernel(q, k, v, mask, Wq, bq, Wk, bk, Wv, bv, Wo, bo):
    from concourse.bass_utils import run_bass_kernel_spmd

    q = np.ascontiguousarray(np.asarray(q), dtype=np.float32)
    k = np.ascontiguousarray(np.asarray(k), dtype=np.float32)
    v = np.ascontiguousarray(np.asarray(v), dtype=np.float32)
    Wq, Wk, Wv, Wo = (np.asarray(w, dtype=np.float32) for w in (Wq, Wk, Wv, Wo))
    bq, bk_, bv_, bo = (np.asarray(x, dtype=np.float32) for x in (bq, bk, bv, bo))

    mask_2d = np.asarray(mask).reshape(S, S)
    causal = bool(np.array_equal(mask_2d != 0, np.tril(np.ones((S, S), bool))))
    if not causal:
        return _numpy_fallback(q, k, v, mask, Wq, bq, Wk, bk_, Wv, bv_, Wo, bo)

    nc = _get_program()
    in_maps = make_in_maps(q, k, v, Wq, bq, Wk, bk_, Wv, bv_, Wo)
    res = run_bass_kernel_spmd(nc, in_maps, list(range(NCORES))).results
    return assemble_output(res, bv_, Wo, bo)


if __name__ == "__main__":
    nc = build_program()
    print("program built + compiled OK")


# revision 79
# speedup vs baseline: 1.0064x; 1.0015x over previous
"""Multi-head causal attention (B=4, S=2048, D=1024, H=16) on 8 trn2 NeuronCores.

Sharding: core = (batch b, head-group g), g in 0..1; each core computes heads
g*8..g*8+7 of batch b end-to-end plus its partial output projection; host sums
the two partials per batch and adds (bo + bv @ Wo.T) (the v-bias folds into
the host-side bias since softmax weights sum to 1).

Per-core dataflow, shaped for a PE whose matmul time is
(output free size) x (cycles/row of the moving dtype):
  QKV proj   compensated fp8 DoubleRow: x ~ (x8+dx8)/16, w ~ (w8+dw8)/256
             (host-split pairs), 3 product terms at a uniform 4096x PSUM
             scale -> 0.75x the bf16 row count at ~bf16 accuracy; the
             psum->sbuf copy rescales by 1/4096 and adds the q/k biases.
  scores     q/k stored fp8e4 in a DoubleRow layout (partition =
             (head%4)*32 + dk%32, k-tile dim = dk//32): one 0.5-cyc/row
             matmul per (head, k-tile), causal-tight sq windows.
  exp        Scalar-engine Exp into bf16 eg tiles; a measured-safe subset
             of late-block tiles computes exp on DVE instead via an int16
             Schraudolph (bf16-bits magic constant), balancing engines.
  attn@v     FLIPPED: stationary eg [sk, sq-128], moving vhx [sk, 65]
             (bf16) -> 65 rows/matmul; softmax denominator rides in column
             64 (ones column of vhx); causal masks only touch the 16
             true-diagonal 128x128 triangles.
  normalize  per-sq-partition reciprocal [128,4] + one broadcast multiply.
  transpose  PE transpose attn_out [sq,c]->[c,sq] against a bf16 identity.
  out proj   stationary wo bf16, moving outT bf16; last block's psum
             copies run on the (then idle) Scalar engine.

Measured on the harness input: ~200us/core (TimelineSim), rel err 7.8e-3
vs the fp32 reference (gate 2e-2).
"""

import sys

sys.path.insert(0, "/opt/trn_rl_repo")

import numpy as np

B, S, D, H, DK = 4, 2048, 1024, 16, 64
NCORES = 8
CPG = 512          # channels per core (8 heads)
HPC = 8            # heads per core
NB = 4             # sq blocks of 512
SQB = 512
NDT = D // 128     # 8 d-tiles
NCT = CPG // 128   # 4 c-tiles per core (= head pairs)
NST = S // 128     # 16 s-tiles

_PROGRAM = None

# exp tiles offloaded from Scalar via bf16 Schraudolph; returns None (keep
# Scalar Exp), 'v' (DVE) or 'p' (gpsimd). Subset must have measured-good
# end-to-end error.
def SCHR_SEL(blk, j):
    if blk == 3 and j % 2 == 1:
        return 'v'
    if blk == 2 and j % 4 == 1:
        return 'v'
    return None
# out-projection psum->sbuf copy engine per block ('v' = DVE, 's' = Scalar)
COPY_ENG = {0: 'v', 1: 'v', 2: 's', 3: 's'}


def build_program():
    import concourse.tile as tile
    from concourse import mybir, bacc

    F32 = mybir.dt.float32
    BF16 = mybir.dt.bfloat16
    FP8 = mybir.dt.float8e4
    DR = mybir.MatmulPerfMode.DoubleRow
    AF = mybir.ActivationFunctionType
    MUL = mybir.AluOpType.mult
    ADD = mybir.AluOpType.add

    nc = bacc.Bacc("TRN2", target_bir_lowering=False, debug=False,
                   num_devices=NCORES)

    # x and w arrive as compensated fp8 pairs: x ~ (x8 + dx8)/16,
    # w ~ (w8 + dw8)/256 -> every product term is 4096x the true value and
    # accumulates in one PSUM group; the psum->sbuf copy rescales by 1/4096.
    xq = nc.dram_tensor("xq", [2, D, S], FP8, kind="ExternalInput").ap()
    xk = nc.dram_tensor("xk", [2, D, S], FP8, kind="ExternalInput").ap()
    xv = nc.dram_tensor("xv", [2, D, S], FP8, kind="ExternalInput").ap()
    wq = nc.dram_tensor("wq", [2, D, CPG], FP8, kind="ExternalInput").ap()
    wk = nc.dram_tensor("wk", [2, D, CPG], FP8, kind="ExternalInput").ap()
    wv = nc.dram_tensor("wv", [2, D, CPG], FP8, kind="ExternalInput").ap()
    wo = nc.dram_tensor("wo", [CPG, D], BF16, kind="ExternalInput").ap()
    # bqk f32-as-bf16-bytes (cols 0:16) | maskw (16:144) | ident (144:272)
    cst = nc.dram_tensor("cst", [128, 272], BF16, kind="ExternalInput").ap()
    o = nc.dram_tensor("o", [D, S], F32, kind="ExternalOutput").ap()

    xq_r = xq.rearrange("e (o p) s -> p e o s", p=128)
    xk_r = xk.rearrange("e (o p) s -> p e o s", p=128)
    xv_r = xv.rearrange("e (o p) s -> p e o s", p=128)
    wq_r = wq.rearrange("e (o p) c -> p e o c", p=128)
    wk_r = wk.rearrange("e (o p) c -> p e o c", p=128)
    wv_r = wv.rearrange("e (o p) c -> p e o c", p=128)
    wo_r = wo.rearrange("(o p) c -> p o c", p=128)

    with tile.TileContext(nc) as tc:
        with (
            tc.tile_pool(name="wts", bufs=1) as wts,
            tc.tile_pool(name="kv", bufs=1) as kv,
            tc.tile_pool(name="xstr", bufs=2) as xstr,
            tc.tile_pool(name="egp", bufs=2) as egp,
            tc.tile_pool(name="ab", bufs=2) as abp,
            tc.tile_pool(name="sfp", bufs=3) as sfp,
            tc.tile_pool(name="ps_pg", bufs=2, space="PSUM") as ps_pg,
            tc.tile_pool(name="ps_pv", bufs=2, space="PSUM") as ps_pv,
            tc.tile_pool(name="ps_mm", bufs=2, space="PSUM") as ps_mm,
        ):
            # ---- constants (single DMA) ----
            cst_t = wts.tile([128, 272], BF16)
            nc.sync.dma_start(out=cst_t[:], in_=cst[:])
            bqk_t = cst_t[:, 0:16].bitcast(F32)
            maskw_t = cst_t[:, 16:144]
            ident_t = cst_t[:, 144:272]

            # ---- persistent weights/activations ----
            wq_t = wts.tile([128, 2, NDT, CPG], FP8)
            wk_t = wts.tile([128, 2, NDT, CPG], FP8)
            wv_t = wts.tile([128, 2, NDT, CPG], FP8)
            wo_t = wts.tile([128, NCT, D], BF16)
            # q/k kept fp8 in a DoubleRow-friendly layout: partition =
            # (head%4)*32 + dk%32, dim1 = head-group g2, dim2 = dk//32
            kT_t = kv.tile([128, 2, 2, S], FP8)
            vhx_t = kv.tile([128, NST, HPC, DK + 1], BF16)

            # compensated-fp8 term order: (stationary_e, moving_e)
            TERMS = ((0, 0), (0, 1), (1, 0))

            def qproj(blk, b2s=(0, 1, 2, 3), xq_t=None):
                sq0 = blk * SQB
                if xq_t is None:
                    xq_t = xstr.tile([128, 2, NDT, SQB], FP8, tag="xq",
                                     name="xq_t")
                    nc.sync.dma_start(out=xq_t[:],
                                      in_=xq_r[:, :, :, sq0:sq0 + SQB])
                    qproj.qT = xstr.tile([128, 2, 2, SQB], FP8, tag="qT",
                                         name="qT_t")
                qT_t = qproj.qT
                for b2 in b2s:
                    g2, kh = b2 // 2, b2 % 2
                    pq = ps_mm.tile([128, SQB], F32, tag="mm")
                    for ti, (we, xe) in enumerate(TERMS):
                        for p in range(4):
                            nc.tensor.matmul(
                                pq[:],
                                wq_t[:, we, 2 * p:2 * p + 2,
                                     b2 * 128:(b2 + 1) * 128],
                                xq_t[:, xe, 2 * p:2 * p + 2, :],
                                start=(ti == 0 and p == 0),
                                stop=(ti == 2 and p == 3), perf_mode=DR)
                    nc.vector.tensor_scalar(qT_t[:, g2, kh, :], pq[:],
                                            1.0 / 4096, bqk_t[:, b2:b2 + 1],
                                            MUL, ADD)
                return qT_t, xq_t

            def kproj(blk, b2s=(0, 1, 2, 3), xk_t=None):
                sq0 = blk * SQB
                if xk_t is None:
                    xk_t = xstr.tile([128, 2, NDT, SQB], FP8, tag="xk",
                                     name="xk_t")
                    nc.sync.dma_start(out=xk_t[:],
                                      in_=xk_r[:, :, :, sq0:sq0 + SQB])
                for b2 in b2s:
                    g2, kh = b2 // 2, b2 % 2
                    pk = ps_mm.tile([128, SQB], F32, tag="mm")
                    for ti, (we, xe) in enumerate(TERMS):
                        for p in range(4):
                            nc.tensor.matmul(
                                pk[:],
                                wk_t[:, we, 2 * p:2 * p + 2,
                                     b2 * 128:(b2 + 1) * 128],
                                xk_t[:, xe, 2 * p:2 * p + 2, :],
                                start=(ti == 0 and p == 0),
                                stop=(ti == 2 and p == 3), perf_mode=DR)
                    nc.vector.tensor_scalar(
                        kT_t[:, g2, kh, sq0:sq0 + SQB], pk[:],
                        1.0 / 4096, bqk_t[:, 4 + b2:5 + b2], MUL, ADD)
                return xk_t

            def vproj(st_lo, st_hi, chunk):
                if chunk is not None:
                    xv_t = xstr.tile([128, 2, NDT, SQB], FP8, tag="xv",
                                     name="xv_t")
                    vproj.xv = xv_t
                    nc.sync.dma_start(
                        out=xv_t[:],
                        in_=xv_r[:, :, :, chunk * SQB:(chunk + 1) * SQB])
                xv_t = vproj.xv
                for st in range(st_lo, st_hi):
                    s0 = (st % 4) * 128
                    pv = ps_mm.tile([128, SQB], F32, tag="mm")
                    for ti, (xe, we) in enumerate(TERMS):
                        for p in range(4):
                            nc.tensor.matmul(
                                pv[:],
                                xv_t[:, xe, 2 * p:2 * p + 2, s0:s0 + 128],
                                wv_t[:, we, 2 * p:2 * p + 2, :],
                                start=(ti == 0 and p == 0),
                                stop=(ti == 2 and p == 3), perf_mode=DR)
                    nc.vector.tensor_scalar_mul(
                        vhx_t[:, st, :, 0:DK],
                        pv.rearrange("p (h d) -> p h d", h=HPC), 1.0 / 4096)

            # bf16-bits Schraudolph exp for DVE offload:
            # int16(score * 0.125*log2(e)*128 + (127*128 - 5.09)) ~ bf16 bits
            # of exp(score/8); max rel err ~3.3%, error-neutral end to end
            # when restricted to blk3 odd k-tiles (measured).
            SCHR_A = 0.125 * float(np.log2(np.e)) * 128.0
            SCHR_B = 127.0 * 128.0 - 5.09
            I16 = mybir.dt.int16

            def scores_exp(blk, t, qT_t):
                nsk = 4 * (blk + 1)
                eg = egp.tile([128, NST, 2, SQB], BF16, tag="eg")
                for j in range(nsk):
                    w0 = max(0, (j - 4 * blk) * 128)
                    pg = ps_pg.tile([128, 2, SQB], F32, tag="pg")
                    for hp in range(2):
                        h = 2 * t + hp
                        g2, h4 = h // 4, h % 4
                        nc.tensor.matmul(
                            pg[:, hp, w0:SQB],
                            kT_t[h4 * 32:h4 * 32 + 32, g2, :,
                                 j * 128:(j + 1) * 128],
                            qT_t[h4 * 32:h4 * 32 + 32, g2, :, w0:SQB],
                            start=True, stop=True, perf_mode=DR,
                            tile_position=(h4 * 32, 0))
                    se = SCHR_SEL(blk, j)
                    if se:
                        eng = nc.vector if se == 'v' else nc.gpsimd
                        eng.tensor_scalar(
                            eg[:, j, :, w0:SQB].bitcast(I16),
                            pg[:, :, w0:SQB], SCHR_A, SCHR_B, MUL, ADD)
                    else:
                        nc.scalar.activation(eg[:, j, :, w0:SQB],
                                             pg[:, :, w0:SQB],
                                             AF.Exp, bias=0.0, scale=0.125)
                    if w0 > 0 or j == 4 * blk:
                        nc.vector.tensor_tensor(
                            eg[:, j, :, w0:w0 + 128],
                            eg[:, j, :, w0:w0 + 128],
                            maskw_t.unsqueeze(1).to_broadcast((128, 2, 128)),
                            MUL)
                return eg

            def attnv_norm(blk, t, attn_sb, eg):
                for hp in range(2):
                    h = 2 * t + hp
                    pv = ps_pv.tile([128, 4, DK + 1], F32, tag="pv")
                    for sub in range(4):
                        ig = 4 * blk + sub
                        for j in range(ig + 1):
                            nc.tensor.matmul(
                                pv[:, sub, :],
                                eg[:, j, hp, sub * 128:sub * 128 + 128],
                                vhx_t[:, j, h, :],
                                start=(j == 0), stop=(j == ig))
                    recip = abp.tile([128, 4], F32, tag="recip")
                    nc.vector.reciprocal(recip[:], pv[:, :, DK])
                    nc.vector.tensor_tensor(
                        attn_sb[:, :, h * DK:(h + 1) * DK],
                        pv[:, :, 0:DK],
                        recip.unsqueeze(-1).to_broadcast((128, 4, DK)),
                        MUL)

            def transpose_out(blk, attn_sb):
                outT_sb = abp.tile([128, NCT, SQB], BF16, tag="outT")
                for half in range(2):
                    ptr = ps_mm.tile([128, 8, 128], BF16, tag="mm")
                    for i in range(2):
                        sub = half * 2 + i
                        for ct in range(NCT):
                            nc.tensor.transpose(
                                ptr[:, i * 4 + ct, :],
                                attn_sb[:, sub, ct * 128:(ct + 1) * 128],
                                ident_t[:])
                    dst = outT_sb.rearrange("p c (u q) -> p u c q", u=4)
                    nc.vector.tensor_copy(
                        dst[:, half * 2:half * 2 + 2, :, :],
                        ptr.rearrange("p (i c) q -> p i c q", i=2))
                return outT_sb

            def transpose_part(attn_sb, outT_sb, cts):
                # transpose a subset of head-pair columns (all 4 sq-subs);
                # lets the last block's ct 0..2 run before its final norm
                dst = outT_sb.rearrange("p c (u q) -> p u c q", u=4)
                nct = len(cts)
                for half in range(2):
                    ptr = ps_mm.tile([128, 2 * nct, 128], BF16, tag="mm",
                                     name="ptr")
                    for i in range(2):
                        sub = half * 2 + i
                        for ci, ct in enumerate(cts):
                            nc.tensor.transpose(
                                ptr[:, i * nct + ci, :],
                                attn_sb[:, sub, ct * 128:(ct + 1) * 128],
                                ident_t[:])
                    nc.vector.tensor_copy(
                        dst[:, half * 2:half * 2 + 2, cts[0]:cts[0] + nct,
                            :],
                        ptr.rearrange("p (i c) q -> p i c q", i=2))

            def outproj(blk, outT_sb):
                sq0 = blk * SQB
                for dt_i in range(8):
                    pf = ps_mm.tile([128, SQB], F32, tag="mm")
                    for ct in range(NCT):
                        nc.tensor.matmul(
                            pf[:],
                            wo_t[:, ct, dt_i * 128:(dt_i + 1) * 128],
                            outT_sb[:, ct, :],
                            start=(ct == 0), stop=(ct == NCT - 1))
                    sf = sfp.tile([128, SQB], F32, tag="sf")
                    if COPY_ENG[blk] == 's':
                        nc.scalar.copy(sf[:], pf[:])
                    else:
                        nc.vector.tensor_copy(sf[:], pf[:])
                    nc.sync.dma_start(
                        out=o[dt_i * 128:(dt_i + 1) * 128, sq0:sq0 + SQB],
                        in_=sf[:])

            # ================= schedule =================
            # wq/wk arrive as half-DMAs so the first b2 blocks start early;
            # blk0's projections are interleaved with its first score groups
            # to get the Scalar engine (exp) running as soon as possible.
            nc.sync.dma_start(out=wq_t[:, :, :, 0:256],
                              in_=wq_r[:, :, :, 0:256])
            qT, xq0 = qproj(0, (0, 1))
            nc.sync.dma_start(out=wk_t[:, :, :, 0:256],
                              in_=wk_r[:, :, :, 0:256])
            xk0 = kproj(0, (0, 1))
            nc.sync.dma_start(out=wq_t[:, :, :, 256:512],
                              in_=wq_r[:, :, :, 256:512])
            nc.sync.dma_start(out=wk_t[:, :, :, 256:512],
                              in_=wk_r[:, :, :, 256:512])
            nc.sync.dma_start(out=wv_t[:], in_=wv_r[:, :, :, :])
            nc.vector.memset(vhx_t[:, :, :, DK], 1.0)
            nc.sync.dma_start(out=wo_t[:], in_=wo_r[:])

            # Flat depth-2 pipeline over groups g = 4*blk + t; the next
            # block's first score groups are prefetched into the current
            # block's attnv stream so exp never starves at block boundaries.
            qTs = {0: qT}
            egs = {}
            attn_sbs = {0: abp.tile([128, 4, CPG], BF16, tag="attn",
                                    name="attn_sb0")}
            outTs = {}

            def sc(g):
                blk, t = divmod(g, 4)
                egs[g] = scores_exp(blk, t, qTs[blk])

            def av(g):
                blk, t = divmod(g, 4)
                attnv_norm(blk, t, attn_sbs[blk], egs[g])

            def transpA(blk):
                outTs[blk] = abp.tile([128, NCT, SQB], BF16, tag="outT",
                                      name=f"outT{blk}")
                transpose_part(attn_sbs[blk], outTs[blk], (0, 1, 2))

            def transpB(blk):
                transpose_part(attn_sbs[blk], outTs[blk], (3,))
                if blk + 1 < NB:
                    attn_sbs[blk + 1] = abp.tile(
                        [128, 4, CPG], BF16, tag="attn",
                        name=f"attn_sb{blk + 1}")

            sc(0)
            sc(1)
            # --- blk0 ---
            qproj(0, (2, 3), xq0)
            kproj(0, (2, 3), xk0)
            vproj(0, 4, 0)
            av(0); sc(2)
            vproj(4, 8, 1)
            av(1); sc(3)
            kproj(1)
            av(2)
            qTs[1], _ = qproj(1)
            sc(4)
            transpA(0)
            av(3); transpB(0); sc(5)
            # --- blk1 ---
            av(4); sc(6)
            outproj(0, outTs[0])
            av(5); sc(7)
            kproj(2)
            av(6)
            qTs[2], _ = qproj(2)
            sc(8)
            transpA(1)
            av(7); transpB(1); sc(9)
            # --- blk2 ---
            vproj(8, 12, 2)
            av(8); sc(10)
            outproj(1, outTs[1])
            av(9); sc(11)
            kproj(3)
            av(10)
            qTs[3], _ = qproj(3)
            sc(12)
            transpA(2)
            av(11); transpB(2); sc(13)
            # --- blk3 ---
            vproj(12, 16, 3)
            av(12); sc(14)
            outproj(2, outTs[2])
            av(13); sc(15)
            av(14)
            transpA(3)
            av(15)
            transpB(3)
            outproj(3, outTs[3])

    nc.compile()
    return nc


def _get_program():
    global _PROGRAM
    if _PROGRAM is None:
        _PROGRAM = build_program()
    return _PROGRAM


def _make_maskw():
    p = np.arange(128, dtype=np.int64)[:, None]
    f = np.arange(128, dtype=np.int64)[None, :]
    return (f >= p).astype(np.float32)


def _qk_perm():
    # column order for the DoubleRow-friendly q/k layout:
    # block b2=(g2, dk_half): partition = (head%4)*32 + dk%32
    perm = np.empty(CPG, np.int64)
    for g2 in range(2):
        for kh in range(2):
            for h4 in range(4):
                base = (g2 * 2 + kh) * 128 + h4 * 32
                src = (4 * g2 + h4) * 64 + kh * 32
                perm[base:base + 32] = np.arange(src, src + 32)
    return perm


def _comp8(a):
    """Scaled f32 array -> stacked (high, delta) fp8 pair, a ~ h + d."""
    import ml_dtypes
    E4 = ml_dtypes.float8_e4m3
    h = a.astype(E4)
    d = (a - h.astype(np.float32)).astype(E4)
    return np.ascontiguousarray(np.stack([h, d]))


def make_in_maps(q, k, v, Wq, bq, Wk, bk, Wv, bv, Wo):
    import ml_dtypes
    BF = ml_dtypes.bfloat16
    wqT, wkT, wvT, woT = Wq.T, Wk.T, Wv.T, Wo.T
    mw = _make_maskw()
    perm = _qk_perm()
    xq8 = [_comp8(16.0 * q[b].T) for b in range(B)]
    xk8 = [_comp8(16.0 * k[b].T) for b in range(B)]
    xv8 = [_comp8(16.0 * v[b].T) for b in range(B)]
    in_maps = []
    for core in range(NCORES):
        b, g = core // 2, core % 2
        cs = slice(g * CPG, (g + 1) * CPG)
        bq_p = bq[cs][perm]
        bk_p = bk[cs][perm]
        cst_host = np.zeros((128, 272), BF)
        bqk_f32 = cst_host[:, 0:16].view(np.float32)
        for b2 in range(4):
            bqk_f32[:, b2] = bq_p[b2 * 128:(b2 + 1) * 128]
            bqk_f32[:, 4 + b2] = bk_p[b2 * 128:(b2 + 1) * 128]
        cst_host[:, 16:144] = mw.astype(BF)
        cst_host[:, 144:272] = np.eye(128, dtype=np.float32).astype(BF)
        in_maps.append(dict(
            xq=xq8[b], xk=xk8[b], xv=xv8[b],
            wq=_comp8(256.0 * wqT[:, cs][:, perm]),
            wk=_comp8(256.0 * wkT[:, cs][:, perm]),
            wv=_comp8(256.0 * wvT[:, cs]),
            wo=np.ascontiguousarray(woT[cs, :].astype(BF)),
            cst=cst_host,
        ))
    return in_maps


def assemble_output(results, bv, Wo, bo):
    hb = (bo + bv @ Wo.T).astype(np.float32)
    out = np.empty((B, S, D), np.float32)
    for b in range(B):
        acc = results[2 * b]["o"] + results[2 * b + 1]["o"]  # [D, S]
        out[b] = acc.T + hb[None, :]
    return out


def _numpy_fallback(q, k, v, mask, Wq, bq, Wk, bk, Wv, bv, Wo, bo):
    def split_heads(x):
        return x.reshape(B, S, H, DK).transpose(0, 2, 1, 3)

    qh = split_heads(q @ Wq.T + bq)
    kh = split_heads(k @ Wk.T + bk)
    vh = split_heads(v @ Wv.T + bv)
    out = np.empty((B, H, S, DK), np.float32)
    m = np.broadcast_to(np.asarray(mask).reshape(-1, S, S)[-1], (S, S))
    for b in range(B):
        for h in range(H):
            s = (qh[b, h] @ kh[b, h].T) / np.float32(np.sqrt(DK))
            s = np.where(m == 0, np.float32(-1e9), s)
            s = s - s.max(axis=-1, keepdims=True)
            e = np.exp(s)
            a = e / e.sum(axis=-1, keepdims=True)
            out[b, h] = a @ vh[b, h]
    out = out.transpose(0, 2, 1, 3).reshape(B, S, D)
    return out @ Wo.T + bo


def kernel(q, k, v, mask, Wq, bq, Wk, bk, Wv, bv, Wo, bo):
    from concourse.bass_utils import run_bass_kernel_spmd

    q = np.ascontiguousarray(np.asarray(q), dtype=np.float32)
    k = np.ascontiguousarray(np.asarray(k), dtype=np.float32)
    v = np.ascontiguousarray(np.asarray(v), dtype=np.float32)
    Wq, Wk, Wv, Wo = (np.asarray(w, dtype=np.float32) for w in (Wq, Wk, Wv, Wo))
    bq, bk_, bv_, bo = (np.asarray(x, dtype=np.float32) for x in (bq, bk, bv, bo))

    mask_2d = np.asarray(mask).reshape(S, S)
    causal = bool(np.array_equal(mask_2d != 0, np.tril(np.ones((S, S), bool))))
    if not causal:
        return _numpy_fallback(q, k, v, mask, Wq, bq, Wk, bk_, Wv, bv_, Wo, bo)

    nc = _get_program()
    in_maps = make_in_maps(q, k, v, Wq, bq, Wk, bk_, Wv, bv_, Wo)
    res = run_bass_kernel_spmd(nc, in_maps, list(range(NCORES))).results
    return assemble_output(res, bv_, Wo, bo)


if __name__ == "__main__":
    nc = build_program()
    print("program built + compiled OK")


# revision 80
# speedup vs baseline: 1.0188x; 1.0123x over previous
"""Multi-head causal attention (B=4, S=2048, D=1024, H=16) on 8 trn2 NeuronCores.

Sharding: core = (batch b, head-group g), g in 0..1; each core computes heads
g*8..g*8+7 of batch b end-to-end plus its partial output projection; host sums
the two partials per batch and adds (bo + bv @ Wo.T) (the v-bias folds into
the host-side bias since softmax weights sum to 1).

Per-core dataflow, shaped for a PE whose matmul time is
(output free size) x (cycles/row of the moving dtype):
  QKV proj   compensated fp8 DoubleRow: x ~ (x8+dx8)/16, w ~ (w8+dw8)/256
             (host-split pairs), 3 product terms at a uniform 4096x PSUM
             scale -> 0.75x the bf16 row count at ~bf16 accuracy; the
             psum->sbuf copy rescales by 1/4096 and adds the q/k biases.
  scores     q/k stored fp8e4 in a DoubleRow layout (partition =
             (head%4)*32 + dk%32, k-tile dim = dk//32): one 0.5-cyc/row
             matmul per (head, k-tile), causal-tight sq windows.
  exp        Scalar-engine Exp into bf16 eg tiles; a measured-safe subset
             of late-block tiles computes exp on DVE instead via an int16
             Schraudolph (bf16-bits magic constant), balancing engines.
  attn@v     FLIPPED: stationary eg [sk, sq-128], moving vhx [sk, 65]
             (bf16) -> 65 rows/matmul; softmax denominator rides in column
             64 (ones column of vhx); causal masks only touch the 16
             true-diagonal 128x128 triangles.
  normalize  per-sq-partition reciprocal [128,4] + one broadcast multiply.
  transpose  PE transpose attn_out [sq,c]->[c,sq] against a bf16 identity.
  out proj   stationary wo bf16, moving outT bf16; last block's psum
             copies run on the (then idle) Scalar engine.

Measured on the harness input: ~200us/core (TimelineSim), rel err 7.8e-3
vs the fp32 reference (gate 2e-2).
"""

import sys

sys.path.insert(0, "/opt/trn_rl_repo")

import numpy as np

B, S, D, H, DK = 4, 2048, 1024, 16, 64
NCORES = 8
CPG = 512          # channels per core (8 heads)
HPC = 8            # heads per core
NB = 4             # sq blocks of 512
SQB = 512
NDT = D // 128     # 8 d-tiles
NCT = CPG // 128   # 4 c-tiles per core (= head pairs)
NST = S // 128     # 16 s-tiles

_PROGRAM = None

# exp tiles offloaded from Scalar via bf16 Schraudolph; returns None (keep
# Scalar Exp), 'v' (DVE) or 'p' (gpsimd). Subset must have measured-good
# end-to-end error.
def SCHR_SEL(blk, j):
    if blk == 3 and j % 2 == 1:
        return 'v'
    if blk == 2 and j % 4 == 1:
        return 'v'
    return None
# out-projection psum->sbuf copy engine per block ('v' = DVE, 's' = Scalar)
COPY_ENG = {0: 'v', 1: 'v', 2: 's', 3: 's'}


def build_program():
    import concourse.tile as tile
    from concourse import mybir, bacc

    F32 = mybir.dt.float32
    BF16 = mybir.dt.bfloat16
    FP8 = mybir.dt.float8e4
    DR = mybir.MatmulPerfMode.DoubleRow
    AF = mybir.ActivationFunctionType
    MUL = mybir.AluOpType.mult
    ADD = mybir.AluOpType.add

    nc = bacc.Bacc("TRN2", target_bir_lowering=False, debug=False,
                   num_devices=NCORES)

    # x and w arrive as compensated fp8 pairs: x ~ (x8 + dx8)/16,
    # w ~ (w8 + dw8)/256 -> every product term is 4096x the true value and
    # accumulates in one PSUM group; the psum->sbuf copy rescales by 1/4096.
    xq = nc.dram_tensor("xq", [2, D, S], FP8, kind="ExternalInput").ap()
    xk = nc.dram_tensor("xk", [2, D, S], FP8, kind="ExternalInput").ap()
    xv = nc.dram_tensor("xv", [2, D, S], FP8, kind="ExternalInput").ap()
    wq = nc.dram_tensor("wq", [2, D, CPG], FP8, kind="ExternalInput").ap()
    wk = nc.dram_tensor("wk", [2, D, CPG], FP8, kind="ExternalInput").ap()
    wv = nc.dram_tensor("wv", [2, D, CPG], FP8, kind="ExternalInput").ap()
    wo = nc.dram_tensor("wo", [CPG, D], BF16, kind="ExternalInput").ap()
    # bqk f32-as-bf16-bytes (cols 0:16) | maskw (16:144) | ident (144:272)
    cst = nc.dram_tensor("cst", [128, 272], BF16, kind="ExternalInput").ap()
    o = nc.dram_tensor("o", [D, S], F32, kind="ExternalOutput").ap()

    xq_r = xq.rearrange("e (o p) s -> p e o s", p=128)
    xk_r = xk.rearrange("e (o p) s -> p e o s", p=128)
    xv_r = xv.rearrange("e (o p) s -> p e o s", p=128)
    wq_r = wq.rearrange("e (o p) c -> p e o c", p=128)
    wk_r = wk.rearrange("e (o p) c -> p e o c", p=128)
    wv_r = wv.rearrange("e (o p) c -> p e o c", p=128)
    wo_r = wo.rearrange("(o p) c -> p o c", p=128)

    with tile.TileContext(nc) as tc:
        with (
            tc.tile_pool(name="wts", bufs=1) as wts,
            tc.tile_pool(name="kv", bufs=1) as kv,
            tc.tile_pool(name="xstr", bufs=2) as xstr,
            tc.tile_pool(name="egp", bufs=2) as egp,
            tc.tile_pool(name="ab", bufs=2) as abp,
            tc.tile_pool(name="sfp", bufs=4) as sfp,
            tc.tile_pool(name="ps_pg", bufs=2, space="PSUM") as ps_pg,
            tc.tile_pool(name="ps_pv", bufs=2, space="PSUM") as ps_pv,
            tc.tile_pool(name="ps_mm", bufs=2, space="PSUM") as ps_mm,
        ):
            # ---- constants (single DMA) ----
            cst_t = wts.tile([128, 272], BF16)
            nc.sync.dma_start(out=cst_t[:], in_=cst[:])
            bqk_t = cst_t[:, 0:16].bitcast(F32)
            maskw_t = cst_t[:, 16:144]
            ident_t = cst_t[:, 144:272]

            # ---- persistent weights/activations ----
            wq_t = wts.tile([128, 2, NDT, CPG], FP8)
            wk_t = wts.tile([128, 2, NDT, CPG], FP8)
            wv_t = wts.tile([128, 2, NDT, CPG], FP8)
            wo_t = wts.tile([128, NCT, D], BF16)
            # q/k kept fp8 in a DoubleRow-friendly layout: partition =
            # (head%4)*32 + dk%32, dim1 = head-group g2, dim2 = dk//32
            kT_t = kv.tile([128, 2, 2, S], FP8)
            vhx_t = kv.tile([128, NST, HPC, DK + 1], BF16)

            # compensated-fp8 term order: (stationary_e, moving_e)
            TERMS = ((0, 0), (0, 1), (1, 0))

            def qproj(blk, b2s=(0, 1, 2, 3), xq_t=None):
                sq0 = blk * SQB
                if xq_t is None:
                    xq_t = xstr.tile([128, 2, NDT, SQB], FP8, tag="xq",
                                     name="xq_t")
                    nc.sync.dma_start(out=xq_t[:],
                                      in_=xq_r[:, :, :, sq0:sq0 + SQB])
                    qproj.qT = xstr.tile([128, 2, 2, SQB], FP8, tag="qT",
                                         name="qT_t")
                qT_t = qproj.qT
                for b2 in b2s:
                    g2, kh = b2 // 2, b2 % 2
                    pq = ps_mm.tile([128, SQB], F32, tag="mm")
                    for ti, (we, xe) in enumerate(TERMS):
                        for p in range(4):
                            nc.tensor.matmul(
                                pq[:],
                                wq_t[:, we, 2 * p:2 * p + 2,
                                     b2 * 128:(b2 + 1) * 128],
                                xq_t[:, xe, 2 * p:2 * p + 2, :],
                                start=(ti == 0 and p == 0),
                                stop=(ti == 2 and p == 3), perf_mode=DR)
                    nc.vector.tensor_scalar(qT_t[:, g2, kh, :], pq[:],
                                            1.0 / 4096, bqk_t[:, b2:b2 + 1],
                                            MUL, ADD)
                return qT_t, xq_t

            def kproj(blk, b2s=(0, 1, 2, 3), xk_t=None):
                sq0 = blk * SQB
                if xk_t is None:
                    xk_t = xstr.tile([128, 2, NDT, SQB], FP8, tag="xk",
                                     name="xk_t")
                    nc.sync.dma_start(out=xk_t[:],
                                      in_=xk_r[:, :, :, sq0:sq0 + SQB])
                for b2 in b2s:
                    g2, kh = b2 // 2, b2 % 2
                    pk = ps_mm.tile([128, SQB], F32, tag="mm")
                    for ti, (we, xe) in enumerate(TERMS):
                        for p in range(4):
                            nc.tensor.matmul(
                                pk[:],
                                wk_t[:, we, 2 * p:2 * p + 2,
                                     b2 * 128:(b2 + 1) * 128],
                                xk_t[:, xe, 2 * p:2 * p + 2, :],
                                start=(ti == 0 and p == 0),
                                stop=(ti == 2 and p == 3), perf_mode=DR)
                    nc.vector.tensor_scalar(
                        kT_t[:, g2, kh, sq0:sq0 + SQB], pk[:],
                        1.0 / 4096, bqk_t[:, 4 + b2:5 + b2], MUL, ADD)
                return xk_t

            def vproj(st_lo, st_hi, chunk):
                if chunk is not None:
                    xv_t = xstr.tile([128, 2, NDT, SQB], FP8, tag="xv",
                                     name="xv_t")
                    vproj.xv = xv_t
                    nc.sync.dma_start(
                        out=xv_t[:],
                        in_=xv_r[:, :, :, chunk * SQB:(chunk + 1) * SQB])
                xv_t = vproj.xv
                for st in range(st_lo, st_hi):
                    s0 = (st % 4) * 128
                    pv = ps_mm.tile([128, SQB], F32, tag="mm")
                    for ti, (xe, we) in enumerate(TERMS):
                        for p in range(4):
                            nc.tensor.matmul(
                                pv[:],
                                xv_t[:, xe, 2 * p:2 * p + 2, s0:s0 + 128],
                                wv_t[:, we, 2 * p:2 * p + 2, :],
                                start=(ti == 0 and p == 0),
                                stop=(ti == 2 and p == 3), perf_mode=DR)
                    nc.vector.tensor_scalar_mul(
                        vhx_t[:, st, :, 0:DK],
                        pv.rearrange("p (h d) -> p h d", h=HPC), 1.0 / 4096)

            # bf16-bits Schraudolph exp for DVE offload:
            # int16(score * 0.125*log2(e)*128 + (127*128 - 5.09)) ~ bf16 bits
            # of exp(score/8); max rel err ~3.3%, error-neutral end to end
            # when restricted to blk3 odd k-tiles (measured).
            SCHR_A = 0.125 * float(np.log2(np.e)) * 128.0
            SCHR_B = 127.0 * 128.0 - 5.09
            I16 = mybir.dt.int16

            def scores_exp(blk, t, qT_t):
                nsk = 4 * (blk + 1)
                eg = egp.tile([128, NST, 2, SQB], BF16, tag="eg")
                for j in range(nsk):
                    w0 = max(0, (j - 4 * blk) * 128)
                    pg = ps_pg.tile([128, 2, SQB], F32, tag="pg")
                    for hp in range(2):
                        h = 2 * t + hp
                        g2, h4 = h // 4, h % 4
                        nc.tensor.matmul(
                            pg[:, hp, w0:SQB],
                            kT_t[h4 * 32:h4 * 32 + 32, g2, :,
                                 j * 128:(j + 1) * 128],
                            qT_t[h4 * 32:h4 * 32 + 32, g2, :, w0:SQB],
                            start=True, stop=True, perf_mode=DR,
                            tile_position=(h4 * 32, 0))
                    se = SCHR_SEL(blk, j)
                    if se:
                        eng = nc.vector if se == 'v' else nc.gpsimd
                        eng.tensor_scalar(
                            eg[:, j, :, w0:SQB].bitcast(I16),
                            pg[:, :, w0:SQB], SCHR_A, SCHR_B, MUL, ADD)
                    else:
                        nc.scalar.activation(eg[:, j, :, w0:SQB],
                                             pg[:, :, w0:SQB],
                                             AF.Exp, bias=0.0, scale=0.125)
                    if w0 > 0 or j == 4 * blk:
                        nc.vector.tensor_tensor(
                            eg[:, j, :, w0:w0 + 128],
                            eg[:, j, :, w0:w0 + 128],
                            maskw_t.unsqueeze(1).to_broadcast((128, 2, 128)),
                            MUL)
                return eg

            def attnv_norm(blk, t, attn_sb, eg):
                for hp in range(2):
                    h = 2 * t + hp
                    pv = ps_pv.tile([128, 4, DK + 1], F32, tag="pv")
                    for sub in range(4):
                        ig = 4 * blk + sub
                        for j in range(ig + 1):
                            nc.tensor.matmul(
                                pv[:, sub, :],
                                eg[:, j, hp, sub * 128:sub * 128 + 128],
                                vhx_t[:, j, h, :],
                                start=(j == 0), stop=(j == ig))
                    recip = abp.tile([128, 4], F32, tag="recip")
                    nc.vector.reciprocal(recip[:], pv[:, :, DK])
                    nc.vector.tensor_tensor(
                        attn_sb[:, :, h * DK:(h + 1) * DK],
                        pv[:, :, 0:DK],
                        recip.unsqueeze(-1).to_broadcast((128, 4, DK)),
                        MUL)

            def transpose_out(blk, attn_sb):
                outT_sb = abp.tile([128, NCT, SQB], BF16, tag="outT")
                for half in range(2):
                    ptr = ps_mm.tile([128, 8, 128], BF16, tag="mm")
                    for i in range(2):
                        sub = half * 2 + i
                        for ct in range(NCT):
                            nc.tensor.transpose(
                                ptr[:, i * 4 + ct, :],
                                attn_sb[:, sub, ct * 128:(ct + 1) * 128],
                                ident_t[:])
                    dst = outT_sb.rearrange("p c (u q) -> p u c q", u=4)
                    nc.vector.tensor_copy(
                        dst[:, half * 2:half * 2 + 2, :, :],
                        ptr.rearrange("p (i c) q -> p i c q", i=2))
                return outT_sb

            def transpose_part(attn_sb, outT_sb, cts):
                # transpose a subset of head-pair columns (all 4 sq-subs);
                # lets the last block's ct 0..2 run before its final norm
                dst = outT_sb.rearrange("p c (u q) -> p u c q", u=4)
                nct = len(cts)
                for half in range(2):
                    ptr = ps_mm.tile([128, 2 * nct, 128], BF16, tag="mm",
                                     name="ptr")
                    for i in range(2):
                        sub = half * 2 + i
                        for ci, ct in enumerate(cts):
                            nc.tensor.transpose(
                                ptr[:, i * nct + ci, :],
                                attn_sb[:, sub, ct * 128:(ct + 1) * 128],
                                ident_t[:])
                    nc.vector.tensor_copy(
                        dst[:, half * 2:half * 2 + 2, cts[0]:cts[0] + nct,
                            :],
                        ptr.rearrange("p (i c) q -> p i c q", i=2))

            def outproj(blk, outT_sb):
                sq0 = blk * SQB
                for dt_i in range(8):
                    pf = ps_mm.tile([128, SQB], F32, tag="mm")
                    for ct in range(NCT):
                        nc.tensor.matmul(
                            pf[:],
                            wo_t[:, ct, dt_i * 128:(dt_i + 1) * 128],
                            outT_sb[:, ct, :],
                            start=(ct == 0), stop=(ct == NCT - 1))
                    sf = sfp.tile([128, SQB], F32, tag="sf")
                    if COPY_ENG[blk] == 's':
                        nc.scalar.copy(sf[:], pf[:])
                    else:
                        nc.vector.tensor_copy(sf[:], pf[:])
                    nc.sync.dma_start(
                        out=o[dt_i * 128:(dt_i + 1) * 128, sq0:sq0 + SQB],
                        in_=sf[:])

            # ================= schedule =================
            # wq/wk arrive as half-DMAs so the first b2 blocks start early;
            # blk0's projections are interleaved with its first score groups
            # to get the Scalar engine (exp) running as soon as possible.
            nc.sync.dma_start(out=wq_t[:, :, :, 0:256],
                              in_=wq_r[:, :, :, 0:256])
            qT, xq0 = qproj(0, (0, 1))
            nc.sync.dma_start(out=wk_t[:, :, :, 0:256],
                              in_=wk_r[:, :, :, 0:256])
            xk0 = kproj(0, (0, 1))
            nc.sync.dma_start(out=wq_t[:, :, :, 256:512],
                              in_=wq_r[:, :, :, 256:512])
            nc.sync.dma_start(out=wk_t[:, :, :, 256:512],
                              in_=wk_r[:, :, :, 256:512])
            nc.sync.dma_start(out=wv_t[:], in_=wv_r[:, :, :, :])
            nc.vector.memset(vhx_t[:, :, :, DK], 1.0)
            nc.sync.dma_start(out=wo_t[:], in_=wo_r[:])

            # Flat depth-2 pipeline over groups g = 4*blk + t; the next
            # block's first score groups are prefetched into the current
            # block's attnv stream so exp never starves at block boundaries.
            qTs = {0: qT}
            egs = {}
            attn_sbs = {0: abp.tile([128, 4, CPG], BF16, tag="attn",
                                    name="attn_sb0")}
            outTs = {}

            def sc(g):
                blk, t = divmod(g, 4)
                egs[g] = scores_exp(blk, t, qTs[blk])

            def av(g):
                blk, t = divmod(g, 4)
                attnv_norm(blk, t, attn_sbs[blk], egs[g])

            def transpA(blk):
                outTs[blk] = abp.tile([128, NCT, SQB], BF16, tag="outT",
                                      name=f"outT{blk}")
                transpose_part(attn_sbs[blk], outTs[blk], (0, 1, 2))

            def transpB(blk):
                transpose_part(attn_sbs[blk], outTs[blk], (3,))
                if blk + 1 < NB:
                    attn_sbs[blk + 1] = abp.tile(
                        [128, 4, CPG], BF16, tag="attn",
                        name=f"attn_sb{blk + 1}")

            sc(0)
            sc(1)
            # --- blk0 ---
            qproj(0, (2, 3), xq0)
            kproj(0, (2, 3), xk0)
            vproj(0, 4, 0)
            av(0); sc(2)
            vproj(4, 8, 1)
            av(1); sc(3)
            kproj(1)
            av(2)
            qTs[1], _ = qproj(1)
            sc(4)
            transpA(0)
            av(3); transpB(0); sc(5)
            # --- blk1 ---
            av(4); sc(6)
            outproj(0, outTs[0])
            av(5); sc(7)
            kproj(2)
            av(6)
            qTs[2], _ = qproj(2)
            sc(8)
            transpA(1)
            av(7); transpB(1); sc(9)
            # --- blk2 ---
            vproj(8, 12, 2)
            av(8); sc(10)
            outproj(1, outTs[1])
            av(9); sc(11)
            kproj(3)
            av(10)
            qTs[3], _ = qproj(3)
            sc(12)
            transpA(2)
            av(11); transpB(2); sc(13)
            # --- blk3 ---
            vproj(12, 16, 3)
            av(12); sc(14)
            outproj(2, outTs[2])
            av(13); sc(15)
            av(14)
            transpA(3)
            av(15)
            transpB(3)
            outproj(3, outTs[3])

    nc.compile()
    return nc


def _get_program():
    global _PROGRAM
    if _PROGRAM is None:
        _PROGRAM = build_program()
    return _PROGRAM


def _make_maskw():
    p = np.arange(128, dtype=np.int64)[:, None]
    f = np.arange(128, dtype=np.int64)[None, :]
    return (f >= p).astype(np.float32)


def _qk_perm():
    # column order for the DoubleRow-friendly q/k layout:
    # block b2=(g2, dk_half): partition = (head%4)*32 + dk%32
    perm = np.empty(CPG, np.int64)
    for g2 in range(2):
        for kh in range(2):
            for h4 in range(4):
                base = (g2 * 2 + kh) * 128 + h4 * 32
                src = (4 * g2 + h4) * 64 + kh * 32
                perm[base:base + 32] = np.arange(src, src + 32)
    return perm


def _comp8(a):
    """Scaled f32 array -> stacked (high, delta) fp8 pair, a ~ h + d."""
    import ml_dtypes
    E4 = ml_dtypes.float8_e4m3
    h = a.astype(E4)
    d = (a - h.astype(np.float32)).astype(E4)
    return np.ascontiguousarray(np.stack([h, d]))


def make_in_maps(q, k, v, Wq, bq, Wk, bk, Wv, bv, Wo):
    import ml_dtypes
    BF = ml_dtypes.bfloat16
    wqT, wkT, wvT, woT = Wq.T, Wk.T, Wv.T, Wo.T
    mw = _make_maskw()
    perm = _qk_perm()
    xq8 = [_comp8(16.0 * q[b].T) for b in range(B)]
    xk8 = [_comp8(16.0 * k[b].T) for b in range(B)]
    xv8 = [_comp8(16.0 * v[b].T) for b in range(B)]
    in_maps = []
    for core in range(NCORES):
        b, g = core // 2, core % 2
        cs = slice(g * CPG, (g + 1) * CPG)
        bq_p = bq[cs][perm]
        bk_p = bk[cs][perm]
        cst_host = np.zeros((128, 272), BF)
        bqk_f32 = cst_host[:, 0:16].view(np.float32)
        for b2 in range(4):
            bqk_f32[:, b2] = bq_p[b2 * 128:(b2 + 1) * 128]
            bqk_f32[:, 4 + b2] = bk_p[b2 * 128:(b2 + 1) * 128]
        cst_host[:, 16:144] = mw.astype(BF)
        cst_host[:, 144:272] = np.eye(128, dtype=np.float32).astype(BF)
        in_maps.append(dict(
            xq=xq8[b], xk=xk8[b], xv=xv8[b],
            wq=_comp8(256.0 * wqT[:, cs][:, perm]),
            wk=_comp8(256.0 * wkT[:, cs][:, perm]),
            wv=_comp8(256.0 * wvT[:, cs]),
            wo=np.ascontiguousarray(woT[cs, :].astype(BF)),
            cst=cst_host,
        ))
    return in_maps


def assemble_output(results, bv, Wo, bo):
    hb = (bo + bv @ Wo.T).astype(np.float32)
    out = np.empty((B, S, D), np.float32)
    for b in range(B):
        acc = results[2 * b]["o"] + results[2 * b + 1]["o"]  # [D, S]
        out[b] = acc.T + hb[None, :]
    return out


def _numpy_fallback(q, k, v, mask, Wq, bq, Wk, bk, Wv, bv, Wo, bo):
    def split_heads(x):
        return x.reshape(B, S, H, DK).transpose(0, 2, 1, 3)

    qh = split_heads(q @ Wq.T + bq)
    kh = split_heads(k @ Wk.T + bk)
    vh = split_heads(v @ Wv.T + bv)
    out = np.empty((B, H, S, DK), np.float32)
    m = np.broadcast_to(np.asarray(mask).reshape(-1, S, S)[-1], (S, S))
    for b in range(B):
        for h in range(H):
            s = (qh[b, h] @ kh[b, h].T) / np.float32(np.sqrt(DK))
            s = np.where(m == 0, np.float32(-1e9), s)
            s = s - s.max(axis=-1, keepdims=True)
            e = np.exp(s)
            a = e / e.sum(axis=-1, keepdims=True)
            out[b, h] = a @ vh[b, h]
    out = out.transpose(0, 2, 1, 3).reshape(B, S, D)
    return out @ Wo.T + bo


def kernel(q, k, v, mask, Wq, bq, Wk, bk, Wv, bv, Wo, bo):
    from concourse.bass_utils import run_bass_kernel_spmd

    q = np.ascontiguousarray(np.asarray(q), dtype=np.float32)
    k = np.ascontiguousarray(np.asarray(k), dtype=np.float32)
    v = np.ascontiguousarray(np.asarray(v), dtype=np.float32)
    Wq, Wk, Wv, Wo = (np.asarray(w, dtype=np.float32) for w in (Wq, Wk, Wv, Wo))
    bq, bk_, bv_, bo = (np.asarray(x, dtype=np.float32) for x in (bq, bk, bv, bo))

    mask_2d = np.asarray(mask).reshape(S, S)
    causal = bool(np.array_equal(mask_2d != 0, np.tril(np.ones((S, S), bool))))
    if not causal:
        return _numpy_fallback(q, k, v, mask, Wq, bq, Wk, bk_, Wv, bv_, Wo, bo)

    nc = _get_program()
    in_maps = make_in_maps(q, k, v, Wq, bq, Wk, bk_, Wv, bv_, Wo)
    res = run_bass_kernel_spmd(nc, in_maps, list(range(NCORES))).results
    return assemble_output(res, bv_, Wo, bo)


if __name__ == "__main__":
    nc = build_program()
    print("program built + compiled OK")
